# revision 13
# baseline (speedup 1.0000x reference)
"""Distributed causal attention (RoPE) kernel for one TRN2 chip (8 NeuronCores).

Reference computation (per batch b):
    q = x @ Wq, k = x @ Wk, v = x @ Wv        (E=1024 -> H=16 heads x D=64)
    RoPE on q,k; causal softmax attention per head; out = attn_out @ Wo

Sharding: data-parallel over batch (B=2) x tensor-parallel over heads (4 groups
of 4 heads). Core c = 4*b + g handles batch b, heads [4g, 4g+4).
Each core computes its 4 heads' attention output A_g^T; per-(head-pair,
position-tile) AllGathers assemble the full attn^T while later tiles are still
computing, and each core then computes a disjoint column slice of the output
projection (column-parallel Wo), so the host only concatenates.

Everything on-chip runs transposed ([feature, position]) so the QK^T
contraction (head dim D) and PV contraction (keys) both have their contraction
axis on SBUF partitions. The host pre-transposes x/cos/sin when sharding;
weights stay natural (Wo rows are host-permuted to match the gathered layout).

Compute dtype: bfloat16 operands with fp32 PSUM accumulation.
"""

import sys

for _p in ("/opt/trn_rl_repo",):
    if _p not in sys.path:
        sys.path.insert(0, _p)

import numpy as np
import ml_dtypes

import concourse.bass as bass  # noqa: F401  (import order matters)
import concourse.mybir as mybir
import concourse.tile as tile
from concourse import bacc
from concourse import bass_utils

S = 2048          # sequence length
E = 1024          # embed dim
D = 64            # head dim
HPC = 4           # heads per core
F = HPC * D       # 256 features per core
NT = S // 512     # 4 moving-dim tiles
KC = E // 128     # 8 contraction chunks
PC = S // 128     # 16 position chunks (key chunks)
N_CORES = 8
GROUPS = [[0, 1, 2, 3], [4, 5, 6, 7]]

DT = mybir.dt.bfloat16
NPDT = ml_dtypes.bfloat16
F32 = mybir.dt.float32


def build_nc(debug_taps=False):
    nc = bacc.Bacc(
        "TRN2", target_bir_lowering=False, debug=False, num_devices=N_CORES
    )
    xT = nc.dram_tensor("xT", [E, S], DT, kind="ExternalInput").ap()
    wq = nc.dram_tensor("wq", [128, KC * F], DT, kind="ExternalInput").ap()
    wk = nc.dram_tensor("wk", [128, KC * F], DT, kind="ExternalInput").ap()
    wv = nc.dram_tensor("wv", [128, KC * F], DT, kind="ExternalInput").ap()
    woc = nc.dram_tensor("woc", [128, KC * F], DT, kind="ExternalInput").ap()
    cos2 = nc.dram_tensor("cos2", [128, S], DT, kind="ExternalInput").ap()
    sinm = nc.dram_tensor("sinm", [128, S], DT, kind="ExternalInput").ap()
    out = nc.dram_tensor("out", [F, S], F32, kind="ExternalOutput").ap()

    with tile.TileContext(nc) as tc:
        with (
            tc.tile_pool(name="const", bufs=1) as cp,
            tc.tile_pool(name="work", bufs=3) as wp,
            tc.tile_pool(name="dram", bufs=1, space="DRAM") as dp,
        ):
            x_sb = cp.tile([128, KC, S], DT, tag="xbuf")
            wq_sb = cp.tile([128, KC, F], DT)
            wk_sb = cp.tile([128, KC, F], DT)
            wv_sb = cp.tile([128, KC, F], DT)
            woc_sb = cp.tile([128, KC, F], DT)
            cos_sb = cp.tile([128, S], DT)
            sin_sb = cp.tile([128, S], DT)
            mask_sb = cp.tile([128, 4, 2, 512], DT)
            qT_sb = cp.tile([128, 2, S], DT)
            kT_sb = cp.tile([128, 2, S], DT)
            v_sb = cp.tile([128, PC, HPC, 65], DT)
            a_sb = cp.tile([128, 2, S], DT)  # [64*i+d, pair, pos]
            at_sb = cp.tile([128, KC, S], DT, tag="xbuf")  # reuse x_sb slot
            ot_sb = cp.tile([128, 2, S], F32)

            # per-(pair, j) AllGather bounce buffers
            agin = [
                [dp.tile([128, 512], DT, name=f"agin{p}_{j}", tag=f"agin{p}_{j}") for j in range(NT)]
                for p in range(2)
            ]
            agout = [
                [dp.tile([512, 512], DT, name=f"agout{p}_{j}", tag=f"agout{p}_{j}") for j in range(NT)]
                for p in range(2)
            ]

            # ---- input DMAs (weights first, then x per chunk) ----
            # weights are host-prepacked to the SBUF [p, kc, f] layout so the
            # DMA is fully contiguous
            for w_dram, w_t in ((wq, wq_sb), (wk, wk_sb), (wv, wv_sb), (woc, woc_sb)):
                nc.sync.dma_start(out=w_t[:], in_=w_dram)
            nc.sync.dma_start(out=cos_sb[:], in_=cos2)
            nc.sync.dma_start(out=sin_sb[:], in_=sinm)
            for kc in range(KC):
                nc.sync.dma_start(
                    out=x_sb[:, kc, :], in_=xT[128 * kc : 128 * (kc + 1), :]
                )

            # ---- causal masks (multiplicative, post-exp) ----
            # mask_sb[ki, r, rep, qi] = 1.0 if qi - ki - 128*r >= 0 else 0.0
            nc.gpsimd.memset(mask_sb[:], 1.0)
            for r in range(4):
                nc.gpsimd.affine_select(
                    out=mask_sb[:, r],
                    in_=mask_sb[:, r],
                    compare_op=mybir.AluOpType.is_ge,
                    fill=0.0,
                    base=-128 * r,
                    pattern=[[0, 2], [1, 512]],
                    channel_multiplier=-1,
                )

            # ones column of V (softmax row-sums fall out of the PV matmul)
            nc.vector.memset(v_sb[:, :, :, 64], 1.0)

            # ---- projections: qT/kT (transposed, RoPE'd) and V (natural) ----
            with tc.tile_pool(name="proj_ps", bufs=2, space="PSUM") as pps:

                def qk_slab(w_sb, dstT, s):
                    ps = pps.tile([128, S], F32, tag="proj")
                    for t in range(NT):
                        for kc in range(KC):
                            nc.tensor.matmul(
                                ps[:, 512 * t : 512 * (t + 1)],
                                w_sb[:, kc, 128 * s : 128 * (s + 1)],
                                x_sb[:, kc, 512 * t : 512 * (t + 1)],
                                start=(kc == 0),
                                stop=(kc == KC - 1),
                            )
                    raw = wp.tile([128, S], DT, tag="rope_raw")
                    sw = wp.tile([128, S], DT, tag="rope_sw")
                    tmp = wp.tile([128, S], DT, tag="rope_tmp")
                    nc.vector.tensor_copy(raw[:], ps[:])
                    # rotate_half: rows d<32 of each 64-block read d+32;
                    # rows d>=32 read d-32 (sign pre-baked into sin_sb).
                    # Partition-shifted 1-input copies (2-input ops must be
                    # partition-aligned per the walrus verifier).
                    for blk in range(4):
                        p0 = 32 * blk
                        src = p0 + 32 if blk % 2 == 0 else p0 - 32
                        nc.vector.tensor_copy(
                            sw[p0 : p0 + 32, :], raw[src : src + 32, :]
                        )
                    dst = dstT[:, s, :]
                    nc.vector.tensor_mul(tmp[:], raw[:], cos_sb[:])
                    nc.vector.tensor_mul(dst, sw[:], sin_sb[:])
                    nc.vector.tensor_add(dst, dst, tmp[:])

                def v_chunks(rng):
                    for c in rng:
                        psv = pps.tile([128, F], F32, tag="proj")
                        for kc in range(KC):
                            nc.tensor.matmul(
                                psv[:],
                                x_sb[:, kc, 128 * c : 128 * (c + 1)],
                                wv_sb[:, kc, :],
                                start=(kc == 0),
                                stop=(kc == KC - 1),
                            )
                        nc.vector.tensor_copy(
                            v_sb[:, c, :, 0:64],
                            psv[:].rearrange("p (h d) -> p h d", h=HPC),
                        )

                qk_slab(wq_sb, qT_sb, 0)
                qk_slab(wk_sb, kT_sb, 0)
                v_chunks(range(0, 8))
                qk_slab(wq_sb, qT_sb, 1)
                qk_slab(wk_sb, kT_sb, 1)
                v_chunks(range(8, PC))

            # ---- attention (transposed scores, head-pair packed) ----
            # per (pair, j): after normalization, DMA the [128, 512] slab to
            # the bounce buffer and AllGather it while later tiles compute.
            with (
                tc.tile_pool(name="sc_ps", bufs=2, space="PSUM") as scps,
                tc.tile_pool(name="oa_ps", bufs=4, space="PSUM") as oaps,
            ):
                for pr in range(2):
                    hA, hB = 2 * pr, 2 * pr + 1
                    for j in range(NT):
                        oa = oaps.tile([65, 512], F32, tag="oaug")
                        ob = oaps.tile([65, 512], F32, tag="oaug")
                        nchunks = 4 * j + 4
                        for c in range(nchunks):
                            sc = scps.tile([128, 1024], F32, tag="sc")
                            nc.tensor.matmul(
                                sc[:, 0:512],
                                kT_sb[0:64, pr, 128 * c : 128 * (c + 1)],
                                qT_sb[0:64, pr, 512 * j : 512 * (j + 1)],
                                start=True,
                                stop=True,
                                tile_position=(0, 0),
                            )
                            nc.tensor.matmul(
                                sc[:, 512:1024],
                                kT_sb[64:128, pr, 128 * c : 128 * (c + 1)],
                                qT_sb[64:128, pr, 512 * j : 512 * (j + 1)],
                                start=True,
                                stop=True,
                                tile_position=(64, 0),
                            )
                            pt = wp.tile([128, 1024], DT, tag="p")
                            nc.scalar.activation(
                                pt[:],
                                sc[:],
                                mybir.ActivationFunctionType.Exp,
                                scale=0.125,
                            )
                            if c >= 4 * j:
                                r = c - 4 * j
                                w = 128 * (r + 1)
                                ptv = pt[:].rearrange("p (b n) -> p b n", b=2)
                                nc.vector.tensor_mul(
                                    ptv[:, :, 0:w],
                                    ptv[:, :, 0:w],
                                    mask_sb[:, r, :, 0:w],
                                )
                            nc.tensor.matmul(
                                oa[:],
                                v_sb[:, c, hA, :],
                                pt[:, 0:512],
                                start=(c == 0),
                                stop=(c == nchunks - 1),
                            )
                            nc.tensor.matmul(
                                ob[:],
                                v_sb[:, c, hB, :],
                                pt[:, 512:1024],
                                start=(c == 0),
                                stop=(c == nchunks - 1),
                            )
                        for o_ps, i in ((oa, 0), (ob, 1)):
                            oc = wp.tile([65, 512], F32, tag="oc")
                            rec = wp.tile([1, 512], F32, tag="rec")
                            recr = wp.tile([1, 512], F32, tag="recr")
                            bc = wp.tile([64, 512], F32, tag="bc")
                            nc.vector.tensor_copy(oc[:], o_ps[:])
                            nc.vector.tensor_copy(rec[:], oc[64:65, :])
                            nc.vector.reciprocal_approx_fast(recr[:], rec[:])
                            nc.gpsimd.partition_broadcast(bc[:], recr[:])
                            if i == 0:
                                nc.vector.tensor_mul(
                                    a_sb[0:64, pr, 512 * j : 512 * (j + 1)],
                                    oc[0:64, :],
                                    bc[:],
                                )
                            else:
                                nrm = wp.tile([64, 512], DT, tag="nrm")
                                nc.vector.tensor_mul(nrm[:], oc[0:64, :], bc[:])
                                nc.vector.tensor_copy(
                                    a_sb[64:128, pr, 512 * j : 512 * (j + 1)],
                                    nrm[:],
                                )
                        nc.sync.dma_start(
                            out=agin[pr][j][:],
                            in_=a_sb[:, pr, 512 * j : 512 * (j + 1)],
                        )
                        nc.gpsimd.collective_compute(
                            "AllGather",
                            mybir.AluOpType.bypass,
                            ins=[agin[pr][j].opt()],
                            outs=[agout[pr][j].opt()],
                            replica_groups=GROUPS,
                        )
                        if pr == 1:
                            # stream gathered tiles into SBUF as they land
                            for p2 in range(2):
                                nc.sync.dma_start(
                                    out=at_sb[:, 4 * p2 : 4 * (p2 + 1),
                                              512 * j : 512 * (j + 1)],
                                    in_=agout[p2][j][:].rearrange(
                                        "(rb p) n -> p rb n", p=128
                                    ),
                                )

            # ---- output projection, store ----
            with tc.tile_pool(name="o_ps", bufs=2, space="PSUM") as ops_:
                for j in range(NT):
                    for m in range(2):
                        po = ops_.tile([128, 512], F32, tag="o")
                        for kcc in range(KC):
                            nc.tensor.matmul(
                                po[:],
                                woc_sb[:, kcc, 128 * m : 128 * (m + 1)],
                                at_sb[:, kcc, 512 * j : 512 * (j + 1)],
                                start=(kcc == 0),
                                stop=(kcc == KC - 1),
                            )
                        nc.vector.tensor_copy(
                            ot_sb[:, m, 512 * j : 512 * (j + 1)], po[:]
                        )
                    for m in range(2):
                        nc.sync.dma_start(
                            out=out[128 * m : 128 * (m + 1),
                                    512 * j : 512 * (j + 1)],
                            in_=ot_sb[:, m, 512 * j : 512 * (j + 1)],
                        )

            if debug_taps:
                dq = nc.dram_tensor("dbg_qT", [128, 2, S], DT, kind="ExternalOutput").ap()
                dk = nc.dram_tensor("dbg_kT", [128, 2, S], DT, kind="ExternalOutput").ap()
                dv = nc.dram_tensor("dbg_v", [128, PC, HPC, 65], DT, kind="ExternalOutput").ap()
                da = nc.dram_tensor("dbg_a", [128, 2, S], DT, kind="ExternalOutput").ap()
                dat = nc.dram_tensor("dbg_at", [128, KC, S], DT, kind="ExternalOutput").ap()
                nc.sync.dma_start(out=dq, in_=qT_sb[:])
                nc.sync.dma_start(out=dk, in_=kT_sb[:])
                nc.sync.dma_start(out=dv, in_=v_sb[:])
                nc.sync.dma_start(out=da, in_=a_sb[:])
                nc.sync.dma_start(out=dat, in_=at_sb[:])

    nc.compile()
    return nc


_CACHE = {}


def get_nc():
    if "nc" not in _CACHE:
        _CACHE["nc"] = build_nc()
    return _CACHE["nc"]


def make_in_maps(x, cos, sin, Wq, Wk, Wv, Wo):
    x = np.asarray(x, np.float32)
    cosT = np.asarray(cos, np.float32).T  # [64, S]
    sinT = np.asarray(sin, np.float32).T
    cos2 = np.concatenate([cosT, cosT], axis=0).astype(NPDT)  # [128, S]
    sgn = np.where(np.arange(64)[:, None] < 32, -1.0, 1.0).astype(np.float32)
    sinm1 = sgn * sinT
    sinm = np.concatenate([sinm1, sinm1], axis=0).astype(NPDT)
    # Wo row permutation matching the gathered attn^T layout:
    # gathered e-row order is (pair, rank, head-in-pair, d)
    perm = np.array(
        [
            256 * r + 128 * pr + 64 * i + d0
            for pr in range(2)
            for r in range(4)
            for i in range(2)
            for d0 in range(D)
        ],
        dtype=np.int64,
    )
    def pack(w):
        # [E, F] -> SBUF layout [128, KC*F]: chunk kc = rows [128kc, 128kc+128)
        return np.ascontiguousarray(
            w.reshape(KC, 128, F).transpose(1, 0, 2).reshape(128, KC * F)
        ).astype(NPDT)

    in_maps = []
    for c in range(N_CORES):
        b, g = divmod(c, 4)
        sl = slice(F * g, F * (g + 1))
        Wo_s = np.asarray(Wo, np.float32)[:, sl][perm]
        in_maps.append(
            {
                "xT": np.ascontiguousarray(x[b].T).astype(NPDT),
                "wq": pack(np.asarray(Wq, np.float32)[:, sl]),
                "wk": pack(np.asarray(Wk, np.float32)[:, sl]),
                "wv": pack(np.asarray(Wv, np.float32)[:, sl]),
                "woc": pack(Wo_s),
                "cos2": cos2,
                "sinm": sinm,
            }
        )
    return in_maps


def assemble(results):
    out = np.empty((2, S, E), np.float32)
    for b in range(2):
        otb = np.concatenate(
            [results[4 * b + g]["out"] for g in range(4)], axis=0
        )  # [E, S]
        out[b] = otb.T
    return out


def run(inputs, trace=False, tmpdir=None):
    nc = get_nc()
    in_maps = make_in_maps(**inputs)
    res = bass_utils.run_bass_kernel_spmd(
        nc, in_maps, core_ids=list(range(N_CORES)), trace=trace, tmpdir=tmpdir
    )
    return assemble(res.results), res


def kernel(x, cos, sin, Wq, Wk, Wv, Wo):
    out, _ = run(dict(x=x, cos=cos, sin=sin, Wq=Wq, Wk=Wk, Wv=Wv, Wo=Wo))
    return out


# revision 14
# speedup vs baseline: 1.2015x; 1.2015x over previous
"""Distributed causal attention (RoPE) kernel for one TRN2 chip (8 NeuronCores).

Reference computation (per batch b):
    q = x @ Wq, k = x @ Wk, v = x @ Wv        (E=1024 -> H=16 heads x D=64)
    RoPE on q,k; causal softmax attention per head; out = attn_out @ Wo

Sharding: data-parallel over batch (B=2) x tensor-parallel over heads (4 groups
of 4 heads). Core c = 4*b + g handles batch b, heads [4g, 4g+4).
Each core computes its 4 heads' attention output A_g^T; per-(head-pair,
position-tile) AllGathers assemble the full attn^T while later tiles are still
computing, and each core then computes a disjoint column slice of the output
projection (column-parallel Wo), so the host only concatenates.

Everything on-chip runs transposed ([feature, position]) so the QK^T
contraction (head dim D) and PV contraction (keys) both have their contraction
axis on SBUF partitions. The host pre-transposes x/cos/sin when sharding;
weights stay natural (Wo rows are host-permuted to match the gathered layout).

Compute dtype: bfloat16 operands with fp32 PSUM accumulation.
"""

import sys

for _p in ("/opt/trn_rl_repo",):
    if _p not in sys.path:
        sys.path.insert(0, _p)

import numpy as np
import ml_dtypes

import concourse.bass as bass  # noqa: F401  (import order matters)
import concourse.mybir as mybir
import concourse.tile as tile
from concourse import bacc
from concourse import bass_utils

S = 2048          # sequence length
E = 1024          # embed dim
D = 64            # head dim
HPC = 4           # heads per core
F = HPC * D       # 256 features per core
NT = S // 512     # 4 moving-dim tiles
KC = E // 128     # 8 contraction chunks
PC = S // 128     # 16 position chunks (key chunks)
N_CORES = 8
GROUPS = [[0, 1, 2, 3], [4, 5, 6, 7]]

DT = mybir.dt.bfloat16
NPDT = ml_dtypes.bfloat16
F32 = mybir.dt.float32


def build_nc(debug_taps=False):
    nc = bacc.Bacc(
        "TRN2", target_bir_lowering=False, debug=False, num_devices=N_CORES
    )
    xT = nc.dram_tensor("xT", [E, S], DT, kind="ExternalInput").ap()
    wq = nc.dram_tensor("wq", [128, KC * F], DT, kind="ExternalInput").ap()
    wk = nc.dram_tensor("wk", [128, KC * F], DT, kind="ExternalInput").ap()
    wv = nc.dram_tensor("wv", [128, KC * F], DT, kind="ExternalInput").ap()
    woc = nc.dram_tensor("woc", [128, KC * F], DT, kind="ExternalInput").ap()
    cos2 = nc.dram_tensor("cos2", [128, S], DT, kind="ExternalInput").ap()
    sinm = nc.dram_tensor("sinm", [128, S], DT, kind="ExternalInput").ap()
    out = nc.dram_tensor("out", [F, S], F32, kind="ExternalOutput").ap()

    with tile.TileContext(nc) as tc:
        with (
            tc.tile_pool(name="const", bufs=1) as cp,
            tc.tile_pool(name="work", bufs=3) as wp,
            tc.tile_pool(name="dram", bufs=1, space="DRAM") as dp,
        ):
            x_sb = cp.tile([128, KC, S], DT, tag="xbuf")
            wq_sb = cp.tile([128, KC, F], DT)
            wk_sb = cp.tile([128, KC, F], DT)
            wv_sb = cp.tile([128, KC, F], DT)
            woc_sb = cp.tile([128, KC, F], DT)
            cos_sb = cp.tile([128, S], DT)
            sin_sb = cp.tile([128, S], DT)
            mask_sb = cp.tile([128, 4, 2, 512], DT)
            qT_sb = cp.tile([128, 2, S], DT)
            kT_sb = cp.tile([128, 2, S], DT)
            v_sb = cp.tile([128, PC, HPC, 65], DT)
            a_sb = cp.tile([128, 2, S], DT)  # [64*i+d, pair, pos]
            at_sb = cp.tile([128, KC, S], DT, tag="xbuf")  # reuse x_sb slot
            ot_sb = cp.tile([128, 2, S], F32)

            # per-(pair, j) AllGather bounce buffers
            agin = [
                [dp.tile([128, 512], DT, name=f"agin{p}_{j}", tag=f"agin{p}_{j}") for j in range(NT)]
                for p in range(2)
            ]
            agout = [
                [dp.tile([512, 512], DT, name=f"agout{p}_{j}", tag=f"agout{p}_{j}") for j in range(NT)]
                for p in range(2)
            ]

            # ---- input DMAs (weights first, then x per chunk) ----
            # weights are host-prepacked to the SBUF [p, kc, f] layout so the
            # DMA is fully contiguous
            for w_dram, w_t in ((wq, wq_sb), (wk, wk_sb), (wv, wv_sb), (woc, woc_sb)):
                nc.sync.dma_start(out=w_t[:], in_=w_dram)
            nc.sync.dma_start(out=cos_sb[:], in_=cos2)
            nc.sync.dma_start(out=sin_sb[:], in_=sinm)
            for kc in range(KC):
                nc.sync.dma_start(
                    out=x_sb[:, kc, :], in_=xT[128 * kc : 128 * (kc + 1), :]
                )

            # ---- causal masks (multiplicative, post-exp) ----
            # mask_sb[ki, r, rep, qi] = 1.0 if qi - ki - 128*r >= 0 else 0.0
            nc.gpsimd.memset(mask_sb[:], 1.0)
            for r in range(4):
                nc.gpsimd.affine_select(
                    out=mask_sb[:, r],
                    in_=mask_sb[:, r],
                    compare_op=mybir.AluOpType.is_ge,
                    fill=0.0,
                    base=-128 * r,
                    pattern=[[0, 2], [1, 512]],
                    channel_multiplier=-1,
                )

            # ones column of V (softmax row-sums fall out of the PV matmul)
            nc.vector.memset(v_sb[:, :, :, 64], 1.0)

            # ---- projections: qT/kT (transposed, RoPE'd) and V (natural) ----
            with tc.tile_pool(name="proj_ps", bufs=2, space="PSUM") as pps:

                def qk_slab(w_sb, dstT, s):
                    ps = pps.tile([128, S], F32, tag="proj")
                    for t in range(NT):
                        for kc in range(KC):
                            nc.tensor.matmul(
                                ps[:, 512 * t : 512 * (t + 1)],
                                w_sb[:, kc, 128 * s : 128 * (s + 1)],
                                x_sb[:, kc, 512 * t : 512 * (t + 1)],
                                start=(kc == 0),
                                stop=(kc == KC - 1),
                            )
                    raw = wp.tile([128, S], DT, tag="rope_raw")
                    sw = wp.tile([128, S], DT, tag="rope_sw")
                    tmp = wp.tile([128, S], DT, tag="rope_tmp")
                    nc.vector.tensor_copy(raw[:], ps[:])
                    # rotate_half: rows d<32 of each 64-block read d+32;
                    # rows d>=32 read d-32 (sign pre-baked into sin_sb).
                    # Partition-shifted 1-input copies (2-input ops must be
                    # partition-aligned per the walrus verifier).
                    for blk in range(4):
                        p0 = 32 * blk
                        src = p0 + 32 if blk % 2 == 0 else p0 - 32
                        nc.vector.tensor_copy(
                            sw[p0 : p0 + 32, :], raw[src : src + 32, :]
                        )
                    dst = dstT[:, s, :]
                    nc.vector.tensor_mul(tmp[:], raw[:], cos_sb[:])
                    nc.vector.tensor_mul(dst, sw[:], sin_sb[:])
                    nc.vector.tensor_add(dst, dst, tmp[:])

                def v_chunks(rng):
                    for c in rng:
                        psv = pps.tile([128, F], F32, tag="proj")
                        for kc in range(KC):
                            nc.tensor.matmul(
                                psv[:],
                                x_sb[:, kc, 128 * c : 128 * (c + 1)],
                                wv_sb[:, kc, :],
                                start=(kc == 0),
                                stop=(kc == KC - 1),
                            )
                        nc.vector.tensor_copy(
                            v_sb[:, c, :, 0:64],
                            psv[:].rearrange("p (h d) -> p h d", h=HPC),
                        )

                qk_slab(wq_sb, qT_sb, 0)
                qk_slab(wk_sb, kT_sb, 0)
                v_chunks(range(0, 8))
                qk_slab(wq_sb, qT_sb, 1)
                qk_slab(wk_sb, kT_sb, 1)
                v_chunks(range(8, PC))

            # ---- attention (transposed scores, head-pair packed) ----
            # per (pair, j): after normalization, DMA the [128, 512] slab to
            # the bounce buffer and AllGather it while later tiles compute.
            with (
                tc.tile_pool(name="sc_ps", bufs=2, space="PSUM") as scps,
                tc.tile_pool(name="oa_ps", bufs=4, space="PSUM") as oaps,
            ):
                for pr in range(2):
                    hA, hB = 2 * pr, 2 * pr + 1
                    # largest tile first so the final AllGather is the small one
                    for j in reversed(range(NT)):
                        oa = oaps.tile([65, 512], F32, tag="oaug")
                        ob = oaps.tile([65, 512], F32, tag="oaug")
                        nchunks = 4 * j + 4
                        for c in range(nchunks):
                            sc = scps.tile([128, 1024], F32, tag="sc")
                            nc.tensor.matmul(
                                sc[:, 0:512],
                                kT_sb[0:64, pr, 128 * c : 128 * (c + 1)],
                                qT_sb[0:64, pr, 512 * j : 512 * (j + 1)],
                                start=True,
                                stop=True,
                                tile_position=(0, 0),
                            )
                            nc.tensor.matmul(
                                sc[:, 512:1024],
                                kT_sb[64:128, pr, 128 * c : 128 * (c + 1)],
                                qT_sb[64:128, pr, 512 * j : 512 * (j + 1)],
                                start=True,
                                stop=True,
                                tile_position=(64, 0),
                            )
                            pt = wp.tile([128, 1024], DT, tag="p")
                            nc.scalar.activation(
                                pt[:],
                                sc[:],
                                mybir.ActivationFunctionType.Exp,
                                scale=0.125,
                            )
                            if c >= 4 * j:
                                r = c - 4 * j
                                w = 128 * (r + 1)
                                ptv = pt[:].rearrange("p (b n) -> p b n", b=2)
                                nc.vector.tensor_mul(
                                    ptv[:, :, 0:w],
                                    ptv[:, :, 0:w],
                                    mask_sb[:, r, :, 0:w],
                                )
                            nc.tensor.matmul(
                                oa[:],
                                v_sb[:, c, hA, :],
                                pt[:, 0:512],
                                start=(c == 0),
                                stop=(c == nchunks - 1),
                            )
                            nc.tensor.matmul(
                                ob[:],
                                v_sb[:, c, hB, :],
                                pt[:, 512:1024],
                                start=(c == 0),
                                stop=(c == nchunks - 1),
                            )
                        for o_ps, i in ((oa, 0), (ob, 1)):
                            oc = wp.tile([65, 512], F32, tag="oc")
                            rec = wp.tile([1, 512], F32, tag="rec")
                            recr = wp.tile([1, 512], F32, tag="recr")
                            bc = wp.tile([64, 512], F32, tag="bc")
                            nc.vector.tensor_copy(oc[:], o_ps[:])
                            nc.vector.tensor_copy(rec[:], oc[64:65, :])
                            nc.vector.reciprocal_approx_fast(recr[:], rec[:])
                            nc.gpsimd.partition_broadcast(bc[:], recr[:])
                            if i == 0:
                                nc.vector.tensor_mul(
                                    a_sb[0:64, pr, 512 * j : 512 * (j + 1)],
                                    oc[0:64, :],
                                    bc[:],
                                )
                            else:
                                nrm = wp.tile([64, 512], DT, tag="nrm")
                                nc.vector.tensor_mul(nrm[:], oc[0:64, :], bc[:])
                                nc.vector.tensor_copy(
                                    a_sb[64:128, pr, 512 * j : 512 * (j + 1)],
                                    nrm[:],
                                )
                        nc.sync.dma_start(
                            out=agin[pr][j][:],
                            in_=a_sb[:, pr, 512 * j : 512 * (j + 1)],
                        )
                        nc.gpsimd.collective_compute(
                            "AllGather",
                            mybir.AluOpType.bypass,
                            ins=[agin[pr][j].opt()],
                            outs=[agout[pr][j].opt()],
                            replica_groups=GROUPS,
                        )

            # ---- gather results in, output projection, store ----
            # (after the attention loop so no engine queue ever stalls on an
            # in-flight collective; j descending matches AG completion order)
            for j in reversed(range(NT)):
                for p2 in range(2):
                    nc.sync.dma_start(
                        out=at_sb[:, 4 * p2 : 4 * (p2 + 1),
                                  512 * j : 512 * (j + 1)],
                        in_=agout[p2][j][:].rearrange(
                            "(rb p) n -> p rb n", p=128
                        ),
                    )
            with tc.tile_pool(name="o_ps", bufs=2, space="PSUM") as ops_:
                for j in reversed(range(NT)):
                    for m in range(2):
                        po = ops_.tile([128, 512], F32, tag="o")
                        for kcc in range(KC):
                            nc.tensor.matmul(
                                po[:],
                                woc_sb[:, kcc, 128 * m : 128 * (m + 1)],
                                at_sb[:, kcc, 512 * j : 512 * (j + 1)],
                                start=(kcc == 0),
                                stop=(kcc == KC - 1),
                            )
                        nc.vector.tensor_copy(
                            ot_sb[:, m, 512 * j : 512 * (j + 1)], po[:]
                        )
                    for m in range(2):
                        nc.sync.dma_start(
                            out=out[128 * m : 128 * (m + 1),
                                    512 * j : 512 * (j + 1)],
                            in_=ot_sb[:, m, 512 * j : 512 * (j + 1)],
                        )

            if debug_taps:
                dq = nc.dram_tensor("dbg_qT", [128, 2, S], DT, kind="ExternalOutput").ap()
                dk = nc.dram_tensor("dbg_kT", [128, 2, S], DT, kind="ExternalOutput").ap()
                dv = nc.dram_tensor("dbg_v", [128, PC, HPC, 65], DT, kind="ExternalOutput").ap()
                da = nc.dram_tensor("dbg_a", [128, 2, S], DT, kind="ExternalOutput").ap()
                dat = nc.dram_tensor("dbg_at", [128, KC, S], DT, kind="ExternalOutput").ap()
                nc.sync.dma_start(out=dq, in_=qT_sb[:])
                nc.sync.dma_start(out=dk, in_=kT_sb[:])
                nc.sync.dma_start(out=dv, in_=v_sb[:])
                nc.sync.dma_start(out=da, in_=a_sb[:])
                nc.sync.dma_start(out=dat, in_=at_sb[:])

    nc.compile()
    return nc


_CACHE = {}


def get_nc():
    if "nc" not in _CACHE:
        _CACHE["nc"] = build_nc()
    return _CACHE["nc"]


def make_in_maps(x, cos, sin, Wq, Wk, Wv, Wo):
    x = np.asarray(x, np.float32)
    cosT = np.asarray(cos, np.float32).T  # [64, S]
    sinT = np.asarray(sin, np.float32).T
    cos2 = np.concatenate([cosT, cosT], axis=0).astype(NPDT)  # [128, S]
    sgn = np.where(np.arange(64)[:, None] < 32, -1.0, 1.0).astype(np.float32)
    sinm1 = sgn * sinT
    sinm = np.concatenate([sinm1, sinm1], axis=0).astype(NPDT)
    # Wo row permutation matching the gathered attn^T layout:
    # gathered e-row order is (pair, rank, head-in-pair, d)
    perm = np.array(
        [
            256 * r + 128 * pr + 64 * i + d0
            for pr in range(2)
            for r in range(4)
            for i in range(2)
            for d0 in range(D)
        ],
        dtype=np.int64,
    )
    def pack(w):
        # [E, F] -> SBUF layout [128, KC*F]: chunk kc = rows [128kc, 128kc+128)
        return np.ascontiguousarray(
            w.reshape(KC, 128, F).transpose(1, 0, 2).reshape(128, KC * F)
        ).astype(NPDT)

    in_maps = []
    for c in range(N_CORES):
        b, g = divmod(c, 4)
        sl = slice(F * g, F * (g + 1))
        Wo_s = np.asarray(Wo, np.float32)[:, sl][perm]
        in_maps.append(
            {
                "xT": np.ascontiguousarray(x[b].T).astype(NPDT),
                "wq": pack(np.asarray(Wq, np.float32)[:, sl]),
                "wk": pack(np.asarray(Wk, np.float32)[:, sl]),
                "wv": pack(np.asarray(Wv, np.float32)[:, sl]),
                "woc": pack(Wo_s),
                "cos2": cos2,
                "sinm": sinm,
            }
        )
    return in_maps


def assemble(results):
    out = np.empty((2, S, E), np.float32)
    for b in range(2):
        otb = np.concatenate(
            [results[4 * b + g]["out"] for g in range(4)], axis=0
        )  # [E, S]
        out[b] = otb.T
    return out


def run(inputs, trace=False, tmpdir=None):
    nc = get_nc()
    in_maps = make_in_maps(**inputs)
    res = bass_utils.run_bass_kernel_spmd(
        nc, in_maps, core_ids=list(range(N_CORES)), trace=trace, tmpdir=tmpdir
    )
    return assemble(res.results), res


def kernel(x, cos, sin, Wq, Wk, Wv, Wo):
    out, _ = run(dict(x=x, cos=cos, sin=sin, Wq=Wq, Wk=Wk, Wv=Wv, Wo=Wo))
    return out
